# revision 2
# baseline (speedup 1.0000x reference)
"""MoE (8-expert top-2 SwiGLU + shared MLP) Trainium2 kernel, 8-core data-parallel.

Data-parallel over the 8192 tokens with HOST-BALANCED token->core assignment:
tokens are sorted by their (top1, top2) expert pair and dealt round-robin to
cores, so per-core per-expert counts track global/8 and one capacity profile
(caps = max count + slack, rounded to 16) serves all cores with little waste.

Per core: router (fp16 matmul + fp32 softmax + top-2 via top-8 sort),
positions via triangular-ones matmul cumsum, then two indirect scatters build
per-slot tables in DRAM: invtab (slot -> local token id, int16, host-init to a
zero dump row past TC) and wtab (slot -> combine weight, f32, host-init 0).
Expert dispatch is dma_gather(transpose=True) from DRAM x rows.

Combine is fold-and-scatter: the PSUM->SBUF copy of each expert GEMM2 tile is
an ACTIVATE with per-partition scale = the slot's combine weight (from wtab),
and the scaled tile is dma_scatter_add-ed (out[idx,:] += in) straight into the
output rows. The shared-MLP GEMM2 runs FIRST (right after shared GEMM1) and
plain-stores shared/3 into out, so the adds land on top; its PE time covers
the routing/dispatch chain. No ybuf round-trip, no gather tail.

Weight streams are spread across DMA queues (w1/ws1 sync, w3/ws3 vector, w2
halves scalar+vector, out stores scalar, tables+gathers gpsimd, scatter-adds
SWDGE q1) so no queue needs >~150 GB/s. 2/3 (moe) and 1/3 (shared) output
scales are folded into w2/ws2 on host.
"""

import os
import sys
import numpy as np

sys.path.insert(0, "/opt/trn_rl_repo")

import ml_dtypes  # noqa: E402
from concourse import bacc, mybir  # noqa: E402
from concourse.bass import IndirectOffsetOnAxis  # noqa: E402
from concourse.tile import TileContext  # noqa: E402
from concourse.bass_utils import run_bass_kernel_spmd  # noqa: E402

F32 = mybir.dt.float32
I32 = mybir.dt.int32
I16 = mybir.dt.int16
F16 = mybir.dt.float16
AF = mybir.ActivationFunctionType
OP = mybir.AluOpType

DT = F16
NP_DT = np.float16

D = 1024
E = 8
HID = 2048
SH = 2048
NCORES = 8
T = 8192
TC = T // NCORES
NTT = TC // 128   # 8 token tiles / core
NDC = D // 128    # 8
NHC = HID // 128  # 16
CAPPAD = 384      # slot table stride per expert (gather needs %128 idxs)
NIC = CAPPAD // 16  # idx columns per expert in the wrapped int16 layout
CSLACK = 3        # capacity slack over host-measured max count
DW = 512          # GEMM2 moving width
NDQ = D // DW
DUMP = TC         # dump row index (x_tok/out have 128 pad rows at TC..TC+127)

_PROGRAMS = {}


def _build_program(caps):
    caps = list(caps)
    CAPMAX = max(caps)

    nc = bacc.Bacc(num_swdge_queues=2)

    x_tok = nc.declare_dram_parameter("x_tok", [TC + 128, D], DT, isOutput=False)
    x_trp = nc.declare_dram_parameter("x_trp", [128, NDC, TC], DT, isOutput=False)
    wrp = nc.declare_dram_parameter("wrp", [128, NDC, E], DT, isOutput=False)
    # packed weights (see kernel() for host-side layouts)
    w1p = nc.declare_dram_parameter("w1p", [E, 8, 128, NDC, 256], DT, isOutput=False)
    w3p = nc.declare_dram_parameter("w3p", [E, 8, 128, NDC, 256], DT, isOutput=False)
    w2p = nc.declare_dram_parameter("w2p", [E, NDQ, 2, 128, 8, DW], DT, isOutput=False)
    ws1p = nc.declare_dram_parameter("ws1p", [8, 128, NDC, 256], DT, isOutput=False)
    ws3p = nc.declare_dram_parameter("ws3p", [8, 128, NDC, 256], DT, isOutput=False)
    ws2p = nc.declare_dram_parameter("ws2p", [NDQ, 128, NHC, DW], DT, isOutput=False)
    cpack = nc.declare_dram_parameter("cpack", [128, 264], F32, isOutput=False)
    tok16 = nc.declare_dram_parameter("tok16", [128, NTT, 16], I16, isOutput=False)
    # slot->token table, host-init to DUMP everywhere; row (e*CAPPAD + c*16+s)
    # holds 16 int16 replicas of the token id of slot c*16+s of expert e
    invtab = nc.declare_dram_parameter("invtab", [E, NIC, 16, 16], I16,
                                       isOutput=False)
    # slot->combine-weight table, host-init 0; row e*CAPPAD+slot = 16 f32 reps
    wtab = nc.declare_dram_parameter("wtab", [E * CAPPAD, 16], F32,
                                     isOutput=False)
    out = nc.declare_dram_parameter("out", [TC + 128, D], F32, isOutput=True)

    inv_rows = invtab.rearrange("e c s r -> (e c s) r")
    wtab_v = wtab.rearrange("(e c p) f -> e p c f", p=128)  # [E,128,3,16]

    with TileContext(nc) as tc:
        with (
            tc.tile_pool(name="const", bufs=1) as cpool,
            tc.tile_pool(name="route", bufs=1) as rpool,
            tc.tile_pool(name="big", bufs=1) as bpool,
            tc.tile_pool(name="wts", bufs=2) as wpool,
            tc.tile_pool(name="work", bufs=2) as kpool,
            tc.tile_pool(name="ps_small", bufs=2, space="PSUM") as ps_s,
            tc.tile_pool(name="ps_uv", bufs=1, space="PSUM") as ps_uv,
            tc.tile_pool(name="ps_y", bufs=4, space="PSUM") as ps_y,
        ):
            # ---- HAM warm-up: dummy matmuls while the first DMAs are in
            # flight so the PE clock is ramped; Silu+Exp act-table preloads
            # ride the same window. Results sunk to DRAM to survive DCE.
            warm_sink = nc.dram_tensor("warm_sink", [128, 512], F32)
            wdum = cpool.tile([128, 512], DT, tag="wdum")
            nc.vector.memset(wdum[:], 0)
            psd = ps_y.tile([128, 512], F32, tag="psy", name="psd_warm")
            for i in range(16):
                nc.tensor.matmul(psd[:], wdum[:, :128], wdum[:],
                                 start=(i == 0), stop=(i == 15))
            wsb = kpool.tile([128, 512], F32, tag="ysb")
            nc.scalar.activation(wsb[:], psd[:], AF.Silu)   # silu table load
            exw = kpool.tile([128, 8], F32, tag="exw")
            nc.scalar.activation(exw[:], psd[:, :8], AF.Exp)  # exp table load
            nc.scalar.dma_start(out=warm_sink[:, :], in_=wsb[:])
            nc.scalar.dma_start(out=warm_sink[:, :8], in_=exw[:])
            # dummy dma_gather so the gpsimd mlp ucode library loads now
            warm_sink2 = nc.dram_tensor("warm_sink2", [128, 8], DT)
            z8 = cpool.tile([128, 8], I16, tag="z8")
            nc.vector.memset(z8[:], 0)
            dxe = kpool.tile([128, NDC, 128], DT, tag="xe", bufs=2)
            nc.gpsimd.dma_gather(
                out_ap=dxe[:], in_ap=x_tok[:, :], idxs_ap=z8[:],
                num_idxs=128, num_idxs_reg=128, elem_size=D, transpose=True)
            nc.sync.dma_start(out=warm_sink2[:, :], in_=dxe[:, 0, 0:8])

            # ---- x^T (host-packed, line-rate) + first weights -------------
            xtr_t = bpool.tile([128, NDC, TC], DT, tag="xbig")
            nc.sync.dma_start(out=xtr_t[:], in_=x_trp[:])
            wr_t = cpool.tile([128, NDC, E], DT, tag="wr")
            nc.sync.dma_start(out=wr_t[:], in_=wrp[:])
            sw1_0 = wpool.tile([128, NDC, 256], DT, tag="w1q", bufs=4, name="sw1_0")
            nc.sync.dma_start(out=sw1_0[:], in_=ws1p[0])
            sw3_0 = wpool.tile([128, NDC, 256], DT, tag="w3q", bufs=4, name="sw3_0")
            nc.vector.dma_start(out=sw3_0[:], in_=ws3p[0])
            # shared GEMM2 weights early on the scalar queue (idle at start)
            w2s_tiles = [wpool.tile([128, NHC, DW], DT, tag="w2s", bufs=2,
                                    name=f"w2s_{dq}") for dq in range(NDQ)]
            for dq in range(NDQ):
                nc.scalar.dma_start(out=w2s_tiles[dq][:], in_=ws2p[dq])

            # ---- resident constants (one packed DMA) ----------------------
            cpk = cpool.tile([128, 264], F32, tag="cpack")
            nc.sync.dma_start(out=cpk[:], in_=cpack[:])
            uts_t = cpk[:, 0:128]
            ones_t = cpk[:, 128:256]
            ecap2_t = cpk[:, 256:264]
            tok16_t = cpool.tile([128, NTT, 16], I16, tag="tok16")
            nc.sync.dma_start(out=tok16_t[:], in_=tok16[:])

            mask_all = rpool.tile([128, NTT, E], F32, tag="mask")
            m1_all = rpool.tile([128, NTT, E], F32, tag="m1")
            t8_all = rpool.tile([128, NTT, 8], F32, tag="t8")
            off2_all = rpool.tile([128, NTT, 2], I32, tag="off2")
            lgacc = rpool.tile([128, NTT, E], F32, tag="lgacc")

            # ---- Router matmuls (fp16 x^T resident) -----------------------
            for tt in range(NTT):
                ps_l = ps_s.tile([128, E], F32, tag="small")
                for dc in range(NDC):
                    nc.tensor.matmul(
                        ps_l[:],
                        xtr_t[:, dc, tt * 128:(tt + 1) * 128],
                        wr_t[:, dc, :],
                        start=(dc == 0), stop=(dc == NDC - 1),
                    )
                nc.scalar.copy(lgacc[:, tt, :], ps_l[:])

            # ---- softmax + top-2 ------------------------------------------
            for tt in range(NTT):
                lg = lgacc[:, tt, :]
                negmx = rpool.tile([128, 1], F32, tag="negmx")
                nc.vector.reduce_max(negmx[:], lg[:], axis=mybir.AxisListType.X,
                                     negate=True)
                ex = rpool.tile([128, E], F32, tag="ex")
                sm = rpool.tile([128, 1], F32, tag="sm")
                nc.scalar.activation(ex[:], lg[:], AF.Exp, bias=negmx[:],
                                     scale=1.0, accum_out=sm[:])
                rcp = rpool.tile([128, 1], F32, tag="rcp")
                nc.vector.reciprocal(rcp[:], sm[:])
                probs = rpool.tile([128, E], F32, tag="probs")
                nc.vector.tensor_scalar_mul(probs[:], ex[:], rcp[:])
                nc.vector.max(t8_all[:, tt, :], probs[:])
                nc.vector.tensor_tensor(
                    out=m1_all[:, tt, :], in0=probs[:],
                    in1=t8_all[:, tt, 0:1].to_broadcast([128, E]),
                    op=OP.is_ge)
                nc.vector.tensor_tensor(
                    out=mask_all[:, tt, :], in0=probs[:],
                    in1=t8_all[:, tt, 1:2].to_broadcast([128, E]),
                    op=OP.is_ge)

            def emit_positions_and_dispatch():
                # positions (cumsum over token tiles); off2 = pos + e*CAPPAD
                # (the slot-table row of the token)
                for tt in range(NTT):
                    ps_p = ps_s.tile([128, E], F32, tag="small")
                    for tp in range(tt):
                        nc.tensor.matmul(ps_p[:], ones_t, mask_all[:, tp, :],
                                         start=(tp == 0), stop=False)
                    nc.tensor.matmul(ps_p[:], uts_t, mask_all[:, tt, :],
                                     start=(tt == 0), stop=True)
                    m2 = rpool.tile([128, E], F32, tag="m2")
                    nc.vector.tensor_sub(m2[:], mask_all[:, tt, :],
                                         m1_all[:, tt, :])
                    sl = rpool.tile([128, E], F32, tag="sl")
                    nc.vector.tensor_add(sl[:], ps_p[:], ecap2_t)
                    s1m = rpool.tile([128, E], F32, tag="s1m")
                    nc.vector.tensor_mul(s1m[:], sl[:], m1_all[:, tt, :])
                    s1f = rpool.tile([128, 1], F32, tag="s1f")
                    nc.vector.reduce_sum(s1f[:], s1m[:],
                                         axis=mybir.AxisListType.X)
                    nc.vector.tensor_copy(off2_all[:, tt, 0:1], s1f[:])
                    s2m = rpool.tile([128, E], F32, tag="s2m")
                    nc.vector.tensor_mul(s2m[:], sl[:], m2[:])
                    s2f = rpool.tile([128, 1], F32, tag="s2f")
                    nc.vector.reduce_sum(s2f[:], s2m[:],
                                         axis=mybir.AxisListType.X)
                    nc.vector.tensor_copy(off2_all[:, tt, 1:2], s2f[:])

                # scatter token ids + combine weights into the slot tables
                for tt in range(NTT):
                    for k in range(2):
                        nc.gpsimd.indirect_dma_start(
                            out=inv_rows[:, :], out_offset=IndirectOffsetOnAxis(
                                ap=off2_all[:, tt, k:k + 1], axis=0),
                            in_=tok16_t[:, tt, :], in_offset=None)
                        w16 = rpool.tile([128, 16], F32, tag="w16")
                        nc.vector.tensor_copy(
                            w16[:], t8_all[:, tt, k:k + 1].to_broadcast([128, 16]))
                        nc.gpsimd.indirect_dma_start(
                            out=wtab[:, :], out_offset=IndirectOffsetOnAxis(
                                ap=off2_all[:, tt, k:k + 1], axis=0),
                            in_=w16[:], in_offset=None)
                # wrapped int16 idx tiles: partition r*16+s, col (e, c) =
                # token of slot c*16+s (replicated for the Q7 cores)
                for r in range(8):
                    nc.gpsimd.dma_start(out=it_all[r * 16:(r + 1) * 16, :, :],
                                        in_=invtab.transpose((2, 3, 0, 1))[:, r])
                # per-slot combine weights in partition layout for GEMM2 scale
                for e in range(E):
                    nc.gpsimd.dma_start(out=wful_all[:, e, :, :],
                                        in_=wtab_v[e])

            it_all = cpool.tile([128, E, NIC], I16, tag="idx")
            wful_all = cpool.tile([128, E, 3, 16], F32, tag="wful")

            # ---- Shared MLP GEMM1 into resident gs_full -------------------
            # (PE streams this while the softmax/positions/scatter routing
            # chain runs on the other engines)
            gs_full = bpool.tile([128, NHC, TC], DT, tag="gshared")
            for hqg in range(8):
                if hqg == 0:
                    wq1, wq3 = sw1_0, sw3_0
                else:
                    wq1 = wpool.tile([128, NDC, 256], DT, tag="w1q", bufs=4)
                    nc.sync.dma_start(out=wq1[:], in_=ws1p[hqg])
                    wq3 = wpool.tile([128, NDC, 256], DT, tag="w3q", bufs=4)
                    nc.vector.dma_start(out=wq3[:], in_=ws3p[hqg])
                for ht in range(2):
                    hg = hqg * 2 + ht
                    for ts in range(2):
                        psu = ps_uv.tile([128, 512], F32, tag="psu")
                        psv = ps_uv.tile([128, 512], F32, tag="psv")
                        for dc in range(NDC):
                            nc.tensor.matmul(
                                psu[:],
                                wq1[:, dc, ht * 128:(ht + 1) * 128],
                                xtr_t[:, dc, ts * 512:(ts + 1) * 512],
                                start=(dc == 0), stop=(dc == NDC - 1))
                        for dc in range(NDC):
                            nc.tensor.matmul(
                                psv[:],
                                wq3[:, dc, ht * 128:(ht + 1) * 128],
                                xtr_t[:, dc, ts * 512:(ts + 1) * 512],
                                start=(dc == 0), stop=(dc == NDC - 1))
                        su = kpool.tile([128, 512], F32, tag="su")
                        nc.scalar.activation(su[:], psu[:], AF.Silu)
                        nc.vector.tensor_mul(
                            gs_full[:, hg, ts * 512:(ts + 1) * 512],
                            su[:], psv[:])
                if hqg == 0:
                    # mask_all is ready by now; run the routing chain so the
                    # expert gathers complete long before the expert GEMMs
                    emit_positions_and_dispatch()

            out_v = out.rearrange("(tt p) d -> p tt d", p=128)

            # ---- Shared MLP GEMM2: plain-store shared/3 into out ----------
            # (runs before the experts so the scatter-adds land on top; PE
            # time here covers the dispatch chain + first expert weight loads)
            for dq in range(NDQ):
                w2s = w2s_tiles[dq]
                for tt in range(NTT):
                    psy = ps_y.tile([128, DW], F32, tag="psy")
                    for hc in range(NHC):
                        nc.tensor.matmul(
                            psy[:],
                            gs_full[:, hc, tt * 128:(tt + 1) * 128],
                            w2s[:, hc, :],
                            start=(hc == 0), stop=(hc == NHC - 1))
                    osb = kpool.tile([128, DW], F32, tag="osb", bufs=4)
                    nc.vector.tensor_copy(osb[:], psy[:])
                    nc.scalar.dma_start(
                        out=out_v[:, tt, dq * DW:(dq + 1) * DW],
                        in_=osb[:])

            # ---- Experts: two halves of 4 ---------------------------------
            EH = E // 2
            for half in range(2):
                g_all = bpool.tile([128, EH, NHC, CAPMAX], DT, tag="g",
                                   name=f"g_all_{half}")
                for ei in range(EH):
                    e = half * EH + ei
                    ce = caps[e]
                    xe_t = kpool.tile([128, NDC, CAPPAD], DT, tag="xe", bufs=2)
                    nc.gpsimd.dma_gather(
                        out_ap=xe_t[:], in_ap=x_tok[:, :],
                        idxs_ap=it_all[:, e, :],
                        num_idxs=CAPPAD, num_idxs_reg=CAPPAD,
                        elem_size=D, transpose=True)

                    for hq in range(8):
                        wq1 = wpool.tile([128, NDC, 256], DT, tag="w1q", bufs=4)
                        nc.sync.dma_start(out=wq1[:], in_=w1p[e, hq])
                        wq3 = wpool.tile([128, NDC, 256], DT, tag="w3q", bufs=4)
                        nc.vector.dma_start(out=wq3[:], in_=w3p[e, hq])
                        for ht in range(2):
                            hg = hq * 2 + ht
                            psu = ps_uv.tile([128, CAPMAX], F32, tag="psu")
                            psv = ps_uv.tile([128, CAPMAX], F32, tag="psv")
                            for dc in range(NDC):
                                nc.tensor.matmul(
                                    psu[:, :ce],
                                    wq1[:, dc, ht * 128:(ht + 1) * 128],
                                    xe_t[:, dc, :ce],
                                    start=(dc == 0), stop=(dc == NDC - 1))
                            for dc in range(NDC):
                                nc.tensor.matmul(
                                    psv[:, :ce],
                                    wq3[:, dc, ht * 128:(ht + 1) * 128],
                                    xe_t[:, dc, :ce],
                                    start=(dc == 0), stop=(dc == NDC - 1))
                            su = kpool.tile([128, CAPMAX], F32, tag="su")
                            nc.scalar.activation(su[:, :ce], psu[:, :ce], AF.Silu)
                            nc.vector.tensor_mul(g_all[:, ei, hg, :ce],
                                                 su[:, :ce], psv[:, :ce])

                # GEMM2 for this half's 4 experts; scaled tiles scatter-add
                # straight into out rows (SWDGE queue 1)
                for ei in range(EH):
                    e = half * EH + ei
                    ce = caps[e]
                    nct = (ce + 127) // 128
                    for dq in range(NDQ):
                        psy_l = [ps_y.tile([128, DW], F32, tag="psy",
                                           name=f"psy_{e}_{dq}_{i}")
                                 for i in range(nct)]
                        for qh in range(2):
                            w2q = wpool.tile([128, 8, DW], DT, tag="w2q")
                            if qh == 0:
                                nc.scalar.dma_start(out=w2q[:], in_=w2p[e, dq, qh])
                            else:
                                nc.vector.dma_start(out=w2q[:], in_=w2p[e, dq, qh])
                            for ct in range(nct):
                                cw = min(128, ce - ct * 128)
                                for hc in range(8):
                                    nc.tensor.matmul(
                                        psy_l[ct][:cw],
                                        g_all[:, ei, qh * 8 + hc,
                                              ct * 128:ct * 128 + cw],
                                        w2q[:, hc, :],
                                        start=(qh == 0 and hc == 0),
                                        stop=(qh == 1 and hc == 7))
                        ysb = kpool.tile([128, 3, DW], F32, tag="ysb", bufs=2)
                        for ct in range(nct):
                            cw = min(128, ce - ct * 128)
                            nc.scalar.activation(
                                ysb[:cw, ct, :], psy_l[ct][:cw], AF.Copy,
                                scale=wful_all[:cw, e, ct, 0:1])
                        nc.gpsimd.dma_scatter_add(
                            out_ap=out[:, dq * DW:(dq + 1) * DW],
                            in_ap=ysb[:, :nct, :],
                            idxs_ap=it_all[:, e, :(ce + 15) // 16],
                            num_idxs=ce, num_idxs_reg=ce,
                            elem_size=DW, elem_step=D,
                            queue_num=1)

    nc.finalize()
    return nc


def _get_program(caps):
    key = tuple(caps)
    if key not in _PROGRAMS:
        _PROGRAMS[key] = _build_program(key)
    return _PROGRAMS[key]


def _pack_w13(w):
    # [E, D, HID] -> [E, hq, p, dc, col] so each (e,hq) load is contiguous
    return np.ascontiguousarray(
        w.reshape(E, NDC, 128, 8, 256).transpose(0, 3, 2, 1, 4).astype(NP_DT))


def _pack_w2(w):
    # [E, HID, D] -> [E, dq, qh, p, hcl, col]
    return np.ascontiguousarray(
        w.reshape(E, 2, 8, 128, NDQ, DW).transpose(0, 4, 1, 3, 2, 5).astype(NP_DT))


def _pack_ws13(w):
    # [D, SH] -> [hqg, p, dc, col]
    return np.ascontiguousarray(
        w.reshape(NDC, 128, 8, 256).transpose(2, 1, 0, 3).astype(NP_DT))


def _pack_ws2(w):
    # [SH, D] -> [dq, p, hc, col]
    return np.ascontiguousarray(
        w.reshape(NHC, 128, NDQ, DW).transpose(2, 1, 0, 3).astype(NP_DT))


def _plan(xf, w_router):
    """fp32 routing + balanced token->core assignment.

    Returns (perm [NCORES, TC] token ids, caps [E]). Tokens are sorted by
    their (top1, top2) expert pair and dealt round-robin, so each core's
    per-expert counts are within a few of global/8.
    """
    logits = xf @ w_router
    part = np.argpartition(-logits, 2, axis=1)[:, :2]
    v = np.take_along_axis(logits, part, axis=1)
    sw = v[:, 0] < v[:, 1]
    part[sw] = part[sw][:, ::-1]
    pair = part[:, 0] * E + part[:, 1]
    order = np.argsort(pair, kind="stable")
    perm = order.reshape(TC, NCORES).T  # core c gets order[c::8]
    counts = np.zeros((NCORES, E), np.int64)
    for c in range(NCORES):
        np.add.at(counts[c], part[perm[c]].ravel(), 1)
    caps = [int(-(-(int(m) + CSLACK) // 16) * 16) for m in counts.max(axis=0)]
    caps = [min(c, CAPPAD) for c in caps]
    return perm, caps


def kernel(x, w_router, w1, w3, w2, ws1, ws3, ws2):
    x = np.asarray(x, dtype=np.float32)
    w_router = np.ascontiguousarray(np.asarray(w_router, dtype=np.float32))
    w1 = np.asarray(w1, dtype=np.float32)
    w3 = np.asarray(w3, dtype=np.float32)
    w2 = np.asarray(w2, dtype=np.float32) * (2.0 / 3.0)
    ws1 = np.asarray(ws1, dtype=np.float32)
    ws3 = np.asarray(ws3, dtype=np.float32)
    ws2 = np.asarray(ws2, dtype=np.float32) * (1.0 / 3.0)

    orig_shape = x.shape
    xf = np.ascontiguousarray(x.reshape(T, D))

    perm, caps = _plan(xf, w_router)

    idx = np.arange(128, dtype=np.float32)
    uts = (idx[:, None] < idx[None, :]).astype(np.float32)
    ones = np.ones((128, 128), dtype=np.float32)
    ecap2 = np.broadcast_to(
        np.arange(E, dtype=np.float32) * CAPPAD, (128, E))
    cpack = np.ascontiguousarray(
        np.concatenate([uts, ones, ecap2], axis=1, dtype=np.float32))
    tok = (np.arange(TC, dtype=np.int16).reshape(NTT, 128).T)[:, :, None]
    tok16 = np.ascontiguousarray(np.broadcast_to(tok, (128, NTT, 16)))
    invtab0 = np.full((E, NIC, 16, 16), DUMP, dtype=np.int16)
    wtab0 = np.zeros((E * CAPPAD, 16), dtype=np.float32)

    w1p, w3p = _pack_w13(w1), _pack_w13(w3)
    w2p = _pack_w2(w2)
    ws1p, ws3p = _pack_ws13(ws1), _pack_ws13(ws3)
    ws2p = _pack_ws2(ws2)
    wrp_h = np.ascontiguousarray(
        w_router.reshape(NDC, 128, E).transpose(1, 0, 2).astype(NP_DT))

    nc = _get_program(caps)

    in_maps = []
    for c in range(NCORES):
        xc = np.ascontiguousarray(xf[perm[c]])
        xtok = np.zeros((TC + 128, D), NP_DT)
        xtok[:TC] = xc.astype(NP_DT)
        xtrp = np.ascontiguousarray(
            xc.T.reshape(NDC, 128, TC).transpose(1, 0, 2).astype(NP_DT))
        in_maps.append({
            "x_tok": xtok, "x_trp": xtrp,
            "wrp": wrp_h,
            "w1p": w1p, "w3p": w3p, "w2p": w2p,
            "ws1p": ws1p, "ws3p": ws3p, "ws2p": ws2p,
            "cpack": cpack, "tok16": tok16,
            "invtab": invtab0, "wtab": wtab0,
        })

    res = run_bass_kernel_spmd(nc, in_maps, list(range(NCORES)))
    out = np.empty((T, D), np.float32)
    for c in range(NCORES):
        out[perm[c]] = res.results[c]["out"][:TC]
    return out.reshape(orig_shape).astype(np.float32)


# revision 5
# speedup vs baseline: 1.0973x; 1.0973x over previous
"""MoE (8-expert top-2 SwiGLU + shared MLP) Trainium2 kernel, 8-core data-parallel.

Data-parallel over the 8192 tokens with HOST-BALANCED token->core assignment:
tokens are sorted by their (top1, top2) expert pair and dealt round-robin to
cores, so per-core per-expert counts track global/8 and one capacity profile
(caps = max count + slack, rounded to 16) serves all cores with little waste.

Per core: router (fp16 matmul + fp32 softmax + top-2 via top-8 sort),
positions via triangular-ones matmul cumsum, then two indirect scatters build
per-slot tables in DRAM: invtab (slot -> local token id, int16, host-init to a
zero dump row past TC) and wtab (slot -> combine weight, f32, host-init 0).
Expert dispatch is dma_gather(transpose=True) from DRAM x rows.

Combine is fold-and-scatter: the PSUM->SBUF copy of each expert GEMM2 tile is
an ACTIVATE with per-partition scale = the slot's combine weight (from wtab),
and the scaled tile is dma_scatter_add-ed (out[idx,:] += in) straight into the
output rows. The shared-MLP GEMM2 runs FIRST (right after shared GEMM1) and
plain-stores shared/3 into out, so the adds land on top; its PE time covers
the routing/dispatch chain. No ybuf round-trip, no gather tail.

Weight streams are spread across DMA queues (w1/ws1 sync, w3/ws3 vector, w2
halves scalar+vector, out stores scalar, tables+gathers gpsimd, scatter-adds
SWDGE q1) so no queue needs >~150 GB/s. 2/3 (moe) and 1/3 (shared) output
scales are folded into w2/ws2 on host.
"""

import os
import sys
import numpy as np

sys.path.insert(0, "/opt/trn_rl_repo")

import ml_dtypes  # noqa: E402
from concourse import bacc, mybir  # noqa: E402
from concourse.bass import IndirectOffsetOnAxis  # noqa: E402
from concourse.tile import TileContext  # noqa: E402
from concourse.bass_utils import run_bass_kernel_spmd  # noqa: E402

F32 = mybir.dt.float32
I32 = mybir.dt.int32
I16 = mybir.dt.int16
F16 = mybir.dt.float16
AF = mybir.ActivationFunctionType
OP = mybir.AluOpType

DT = F16
NP_DT = np.float16

D = 1024
E = 8
HID = 2048
SH = 2048
NCORES = 8
T = 8192
TC = T // NCORES
NTT = TC // 128   # 8 token tiles / core
NDC = D // 128    # 8
NHC = HID // 128  # 16
CAPPAD = 384      # slot table stride per expert (gather needs %128 idxs)
NIC = CAPPAD // 16  # idx columns per expert in the wrapped int16 layout
CSLACK = 2        # capacity slack over host-measured max count
DW = 512          # GEMM2 moving width
NDQ = D // DW
DUMP = TC         # dump row index (x_tok/out have 128 pad rows at TC..TC+127)

_PROGRAMS = {}


def _build_program(caps):
    caps = list(caps)
    CAPMAX = max(caps)

    nc = bacc.Bacc(num_swdge_queues=2)

    x_tok = nc.declare_dram_parameter("x_tok", [TC + 128, D], DT, isOutput=False)
    x_trp = nc.declare_dram_parameter("x_trp", [128, NDC, TC], DT, isOutput=False)
    wrp = nc.declare_dram_parameter("wrp", [128, NDC, E], DT, isOutput=False)
    # packed weights (see kernel() for host-side layouts)
    w1p = nc.declare_dram_parameter("w1p", [E, 8, 128, NDC, 256], DT, isOutput=False)
    w3p = nc.declare_dram_parameter("w3p", [E, 8, 128, NDC, 256], DT, isOutput=False)
    w2p = nc.declare_dram_parameter("w2p", [E, NDQ, 2, 128, 8, DW], DT, isOutput=False)
    ws1p = nc.declare_dram_parameter("ws1p", [8, 128, NDC, 256], DT, isOutput=False)
    ws3p = nc.declare_dram_parameter("ws3p", [8, 128, NDC, 256], DT, isOutput=False)
    ws2p = nc.declare_dram_parameter("ws2p", [NDQ, 128, NHC, DW], DT, isOutput=False)
    cpack = nc.declare_dram_parameter("cpack", [128, 264], F32, isOutput=False)
    tok16 = nc.declare_dram_parameter("tok16", [128, NTT, 16], I16, isOutput=False)
    # slot->token table, host-init to DUMP everywhere; row (e*CAPPAD + c*16+s)
    # holds 16 int16 replicas of the token id of slot c*16+s of expert e
    invtab = nc.declare_dram_parameter("invtab", [E, NIC, 16, 16], I16,
                                       isOutput=False)
    # slot->combine-weight table, host-init 0; row e*CAPPAD+slot = 16 f32 reps
    wtab = nc.declare_dram_parameter("wtab", [E * CAPPAD, 16], F32,
                                     isOutput=False)
    out = nc.declare_dram_parameter("out", [TC + 128, D], F32, isOutput=True)

    inv_rows = invtab.rearrange("e c s r -> (e c s) r")
    wtab_v = wtab.rearrange("(e c p) f -> e p c f", e=E, p=128)  # [E,128,3,16]

    with TileContext(nc) as tc:
        with (
            tc.tile_pool(name="const", bufs=1) as cpool,
            tc.tile_pool(name="route", bufs=1) as rpool,
            tc.tile_pool(name="big", bufs=1) as bpool,
            tc.tile_pool(name="wts", bufs=2) as wpool,
            tc.tile_pool(name="work", bufs=2) as kpool,
            tc.tile_pool(name="ps_small", bufs=2, space="PSUM") as ps_s,
            tc.tile_pool(name="ps_uv", bufs=1, space="PSUM") as ps_uv,
            tc.tile_pool(name="ps_y", bufs=4, space="PSUM") as ps_y,
        ):
            # ---- HAM warm-up: dummy matmuls while the first DMAs are in
            # flight so the PE clock is ramped; Silu+Exp act-table preloads
            # ride the same window. Results sunk to DRAM to survive DCE.
            warm_sink = nc.dram_tensor("warm_sink", [128, 512], F32)
            wdum = cpool.tile([128, 512], DT, tag="wdum")
            nc.vector.memset(wdum[:], 0)
            psd = ps_y.tile([128, 512], F32, tag="psy", name="psd_warm")
            for i in range(16):
                nc.tensor.matmul(psd[:], wdum[:, :128], wdum[:],
                                 start=(i == 0), stop=(i == 15))
            wsb = kpool.tile([128, 512], F32, tag="ysb")
            nc.scalar.activation(wsb[:], psd[:], AF.Silu)   # silu table load
            exw = kpool.tile([128, 8], F32, tag="exw")
            nc.scalar.activation(exw[:], psd[:, :8], AF.Exp)  # exp table load
            nc.scalar.dma_start(out=warm_sink[:, :], in_=wsb[:])
            nc.scalar.dma_start(out=warm_sink[:, :8], in_=exw[:])
            # dummy dma_gather so the gpsimd mlp ucode library loads now
            warm_sink2 = nc.dram_tensor("warm_sink2", [128, 8], DT)
            z8 = cpool.tile([128, 8], I16, tag="z8")
            nc.vector.memset(z8[:], 0)
            dxe = kpool.tile([128, NDC, 128], DT, tag="xe", bufs=2)
            nc.gpsimd.dma_gather(
                out_ap=dxe[:], in_ap=x_tok[:, :], idxs_ap=z8[:],
                num_idxs=128, num_idxs_reg=128, elem_size=D, transpose=True)
            nc.sync.dma_start(out=warm_sink2[:, :], in_=dxe[:, 0, 0:8])

            # ---- x^T (host-packed, line-rate) + first weights -------------
            xtr_t = bpool.tile([128, NDC, TC], DT, tag="xbig")
            nc.sync.dma_start(out=xtr_t[:], in_=x_trp[:])
            wr_t = cpool.tile([128, NDC, E], DT, tag="wr")
            nc.sync.dma_start(out=wr_t[:], in_=wrp[:])
            sw1_0 = wpool.tile([128, NDC, 256], DT, tag="w1q", bufs=4, name="sw1_0")
            nc.sync.dma_start(out=sw1_0[:], in_=ws1p[0])
            sw3_0 = wpool.tile([128, NDC, 256], DT, tag="w3q", bufs=4, name="sw3_0")
            nc.scalar.dma_start(out=sw3_0[:], in_=ws3p[0])
            # shared GEMM2 weights early on the scalar queue (idle at start)
            w2s_tiles = [wpool.tile([128, NHC, DW], DT, tag="w2s", bufs=2,
                                    name=f"w2s_{dq}") for dq in range(NDQ)]
            for dq in range(NDQ):
                nc.scalar.dma_start(out=w2s_tiles[dq][:], in_=ws2p[dq])

            # ---- resident constants (one packed DMA) ----------------------
            cpk = cpool.tile([128, 264], F32, tag="cpack")
            nc.sync.dma_start(out=cpk[:], in_=cpack[:])
            uts_t = cpk[:, 0:128]
            ones_t = cpk[:, 128:256]
            ecap2_t = cpk[:, 256:264]
            tok16_t = cpool.tile([128, NTT, 16], I16, tag="tok16")
            nc.sync.dma_start(out=tok16_t[:], in_=tok16[:])

            mask_all = rpool.tile([128, NTT, E], F32, tag="mask")
            m1_all = rpool.tile([128, NTT, E], F32, tag="m1")
            t8_all = rpool.tile([128, NTT, 8], F32, tag="t8")
            off2_all = rpool.tile([128, NTT, 2], I32, tag="off2")
            lgacc = rpool.tile([128, NTT, E], F32, tag="lgacc")

            # ---- Router matmuls (fp16 x^T resident) -----------------------
            for tt in range(NTT):
                ps_l = ps_s.tile([128, E], F32, tag="small")
                for dc in range(NDC):
                    nc.tensor.matmul(
                        ps_l[:],
                        xtr_t[:, dc, tt * 128:(tt + 1) * 128],
                        wr_t[:, dc, :],
                        start=(dc == 0), stop=(dc == NDC - 1),
                    )
                nc.scalar.copy(lgacc[:, tt, :], ps_l[:])

            # ---- softmax + top-2 ------------------------------------------
            for tt in range(NTT):
                lg = lgacc[:, tt, :]
                negmx = rpool.tile([128, 1], F32, tag="negmx")
                nc.vector.reduce_max(negmx[:], lg[:], axis=mybir.AxisListType.X,
                                     negate=True)
                ex = rpool.tile([128, E], F32, tag="ex")
                sm = rpool.tile([128, 1], F32, tag="sm")
                nc.scalar.activation(ex[:], lg[:], AF.Exp, bias=negmx[:],
                                     scale=1.0, accum_out=sm[:])
                rcp = rpool.tile([128, 1], F32, tag="rcp")
                nc.vector.reciprocal(rcp[:], sm[:])
                probs = rpool.tile([128, E], F32, tag="probs")
                nc.vector.tensor_scalar_mul(probs[:], ex[:], rcp[:])
                nc.vector.max(t8_all[:, tt, :], probs[:])
                nc.vector.tensor_tensor(
                    out=m1_all[:, tt, :], in0=probs[:],
                    in1=t8_all[:, tt, 0:1].to_broadcast([128, E]),
                    op=OP.is_ge)
                nc.vector.tensor_tensor(
                    out=mask_all[:, tt, :], in0=probs[:],
                    in1=t8_all[:, tt, 1:2].to_broadcast([128, E]),
                    op=OP.is_ge)

            def emit_positions_and_dispatch():
                # positions (cumsum over token tiles); off2 = pos + e*CAPPAD
                # (the slot-table row of the token)
                for tt in range(NTT):
                    ps_p = ps_s.tile([128, E], F32, tag="small")
                    for tp in range(tt):
                        nc.tensor.matmul(ps_p[:], ones_t, mask_all[:, tp, :],
                                         start=(tp == 0), stop=False)
                    nc.tensor.matmul(ps_p[:], uts_t, mask_all[:, tt, :],
                                     start=(tt == 0), stop=True)
                    m2 = rpool.tile([128, E], F32, tag="m2")
                    nc.vector.tensor_sub(m2[:], mask_all[:, tt, :],
                                         m1_all[:, tt, :])
                    sl = rpool.tile([128, E], F32, tag="sl")
                    nc.vector.tensor_add(sl[:], ps_p[:], ecap2_t)
                    s1m = rpool.tile([128, E], F32, tag="s1m")
                    nc.vector.tensor_mul(s1m[:], sl[:], m1_all[:, tt, :])
                    s1f = rpool.tile([128, 1], F32, tag="s1f")
                    nc.vector.reduce_sum(s1f[:], s1m[:],
                                         axis=mybir.AxisListType.X)
                    nc.vector.tensor_copy(off2_all[:, tt, 0:1], s1f[:])
                    s2m = rpool.tile([128, E], F32, tag="s2m")
                    nc.vector.tensor_mul(s2m[:], sl[:], m2[:])
                    s2f = rpool.tile([128, 1], F32, tag="s2f")
                    nc.vector.reduce_sum(s2f[:], s2m[:],
                                         axis=mybir.AxisListType.X)
                    nc.vector.tensor_copy(off2_all[:, tt, 1:2], s2f[:])

                # scatter token ids + combine weights into the slot tables
                for tt in range(NTT):
                    for k in range(2):
                        nc.gpsimd.indirect_dma_start(
                            out=inv_rows[:, :], out_offset=IndirectOffsetOnAxis(
                                ap=off2_all[:, tt, k:k + 1], axis=0),
                            in_=tok16_t[:, tt, :], in_offset=None)
                        w16 = rpool.tile([128, 16], F32, tag="w16")
                        nc.vector.tensor_copy(
                            w16[:], t8_all[:, tt, k:k + 1].to_broadcast([128, 16]))
                        nc.gpsimd.indirect_dma_start(
                            out=wtab[:, :], out_offset=IndirectOffsetOnAxis(
                                ap=off2_all[:, tt, k:k + 1], axis=0),
                            in_=w16[:], in_offset=None)
                # wrapped int16 idx tiles: partition r*16+s, col (e, c) =
                # token of slot c*16+s (replicated for the Q7 cores)
                for r in range(8):
                    nc.gpsimd.dma_start(out=it_all[r * 16:(r + 1) * 16, :, :],
                                        in_=invtab.transpose((2, 3, 0, 1))[:, r])
                # per-slot combine weights in partition layout for GEMM2 scale
                for e in range(E):
                    nc.gpsimd.dma_start(out=wful_all[:, e, :, :],
                                        in_=wtab_v[e])

            it_all = cpool.tile([128, E, NIC], I16, tag="idx")
            wful_all = cpool.tile([128, E, 3, 16], F32, tag="wful")

            # ---- Shared MLP GEMM1 into resident gs_full -------------------
            # (PE streams this while the softmax/positions/scatter routing
            # chain runs on the other engines)
            gs_full = bpool.tile([128, NHC, TC], DT, tag="gshared")
            for hqg in range(8):
                if hqg == 0:
                    wq1, wq3 = sw1_0, sw3_0
                else:
                    wq1 = wpool.tile([128, NDC, 256], DT, tag="w1q", bufs=4)
                    nc.sync.dma_start(out=wq1[:], in_=ws1p[hqg])
                    wq3 = wpool.tile([128, NDC, 256], DT, tag="w3q", bufs=4)
                    nc.scalar.dma_start(out=wq3[:], in_=ws3p[hqg])
                for ht in range(2):
                    hg = hqg * 2 + ht
                    for ts in range(2):
                        psu = ps_uv.tile([128, 512], F32, tag="psu")
                        psv = ps_uv.tile([128, 512], F32, tag="psv")
                        for dc in range(NDC):
                            nc.tensor.matmul(
                                psu[:],
                                wq1[:, dc, ht * 128:(ht + 1) * 128],
                                xtr_t[:, dc, ts * 512:(ts + 1) * 512],
                                start=(dc == 0), stop=(dc == NDC - 1))
                        for dc in range(NDC):
                            nc.tensor.matmul(
                                psv[:],
                                wq3[:, dc, ht * 128:(ht + 1) * 128],
                                xtr_t[:, dc, ts * 512:(ts + 1) * 512],
                                start=(dc == 0), stop=(dc == NDC - 1))
                        su = kpool.tile([128, 512], F32, tag="su")
                        nc.scalar.activation(su[:], psu[:], AF.Silu)
                        nc.vector.tensor_mul(
                            gs_full[:, hg, ts * 512:(ts + 1) * 512],
                            su[:], psv[:])
                if hqg == 0:
                    # mask_all is ready by now; run the routing chain so the
                    # expert gathers complete long before the expert GEMMs
                    emit_positions_and_dispatch()

            out_v = out.rearrange("(tt p) d -> p tt d", p=128)

            # ---- Shared MLP GEMM2: plain-store shared/3 into out ----------
            # (runs before the experts so the scatter-adds land on top; PE
            # time here covers the dispatch chain + first expert weight loads)
            for dq in range(NDQ):
                w2s = w2s_tiles[dq]
                for tt in range(NTT):
                    psy = ps_y.tile([128, DW], F32, tag="psy")
                    for hc in range(NHC):
                        nc.tensor.matmul(
                            psy[:],
                            gs_full[:, hc, tt * 128:(tt + 1) * 128],
                            w2s[:, hc, :],
                            start=(hc == 0), stop=(hc == NHC - 1))
                    osb = kpool.tile([128, DW], F32, tag="osb", bufs=4)
                    nc.vector.tensor_copy(osb[:], psy[:])
                    nc.scalar.dma_start(
                        out=out_v[:, tt, dq * DW:(dq + 1) * DW],
                        in_=osb[:])

            # ---- Experts: two halves of 4 ---------------------------------
            EH = E // 2
            for half in range(2):
                g_all = bpool.tile([128, EH, NHC, CAPMAX], DT, tag="g",
                                   name=f"g_all_{half}")
                for ei in range(EH):
                    e = half * EH + ei
                    ce = caps[e]
                    xe_t = kpool.tile([128, NDC, CAPPAD], DT, tag="xe", bufs=2)
                    nc.gpsimd.dma_gather(
                        out_ap=xe_t[:], in_ap=x_tok[:, :],
                        idxs_ap=it_all[:, e, :],
                        num_idxs=CAPPAD, num_idxs_reg=CAPPAD,
                        elem_size=D, transpose=True)

                    for hq in range(8):
                        wq1 = wpool.tile([128, NDC, 256], DT, tag="w1q", bufs=4)
                        nc.sync.dma_start(out=wq1[:], in_=w1p[e, hq])
                        wq3 = wpool.tile([128, NDC, 256], DT, tag="w3q", bufs=4)
                        nc.scalar.dma_start(out=wq3[:], in_=w3p[e, hq])
                        for ht in range(2):
                            hg = hq * 2 + ht
                            psu = ps_uv.tile([128, CAPMAX], F32, tag="psu")
                            psv = ps_uv.tile([128, CAPMAX], F32, tag="psv")
                            for dc in range(NDC):
                                nc.tensor.matmul(
                                    psu[:, :ce],
                                    wq1[:, dc, ht * 128:(ht + 1) * 128],
                                    xe_t[:, dc, :ce],
                                    start=(dc == 0), stop=(dc == NDC - 1))
                            for dc in range(NDC):
                                nc.tensor.matmul(
                                    psv[:, :ce],
                                    wq3[:, dc, ht * 128:(ht + 1) * 128],
                                    xe_t[:, dc, :ce],
                                    start=(dc == 0), stop=(dc == NDC - 1))
                            su = kpool.tile([128, CAPMAX], F32, tag="su")
                            nc.scalar.activation(su[:, :ce], psu[:, :ce], AF.Silu)
                            nc.vector.tensor_mul(g_all[:, ei, hg, :ce],
                                                 su[:, :ce], psv[:, :ce])

                # GEMM2 for this half's 4 experts; scaled tiles scatter-add
                # straight into out rows (SWDGE queue 1)
                for ei in range(EH):
                    e = half * EH + ei
                    ce = caps[e]
                    nct = (ce + 127) // 128
                    for dq in range(NDQ):
                        psy_l = [ps_y.tile([128, DW], F32, tag="psy",
                                           name=f"psy_{e}_{dq}_{i}")
                                 for i in range(nct)]
                        for qh in range(2):
                            w2q = wpool.tile([128, 8, DW], DT, tag="w2q")
                            if qh == 0:
                                nc.scalar.dma_start(out=w2q[:], in_=w2p[e, dq, qh])
                            else:
                                nc.sync.dma_start(out=w2q[:], in_=w2p[e, dq, qh])
                            for ct in range(nct):
                                cw = min(128, ce - ct * 128)
                                for hc in range(8):
                                    nc.tensor.matmul(
                                        psy_l[ct][:cw],
                                        g_all[:, ei, qh * 8 + hc,
                                              ct * 128:ct * 128 + cw],
                                        w2q[:, hc, :],
                                        start=(qh == 0 and hc == 0),
                                        stop=(qh == 1 and hc == 7))
                        ysb = kpool.tile([128, 3, DW], F32, tag="ysb", bufs=2)
                        for ct in range(nct):
                            cw = min(128, ce - ct * 128)
                            nc.scalar.activation(
                                ysb[:cw, ct, :], psy_l[ct][:cw], AF.Copy,
                                scale=wful_all[:cw, e, ct, 0:1])
                        nc.gpsimd.dma_scatter_add(
                            out_ap=out[:, dq * DW:(dq + 1) * DW],
                            in_ap=ysb[:, :nct, :],
                            idxs_ap=it_all[:, e, :(ce + 15) // 16],
                            num_idxs=ce, num_idxs_reg=ce,
                            elem_size=DW, elem_step=D,
                            queue_num=1)

    nc.finalize()
    return nc


def _get_program(caps):
    key = tuple(caps)
    if key not in _PROGRAMS:
        _PROGRAMS[key] = _build_program(key)
    return _PROGRAMS[key]


def _pack_w13(w):
    # [E, D, HID] -> [E, hq, p, dc, col] so each (e,hq) load is contiguous
    return np.ascontiguousarray(
        w.reshape(E, NDC, 128, 8, 256).transpose(0, 3, 2, 1, 4).astype(NP_DT))


def _pack_w2(w):
    # [E, HID, D] -> [E, dq, qh, p, hcl, col]
    return np.ascontiguousarray(
        w.reshape(E, 2, 8, 128, NDQ, DW).transpose(0, 4, 1, 3, 2, 5).astype(NP_DT))


def _pack_ws13(w):
    # [D, SH] -> [hqg, p, dc, col]
    return np.ascontiguousarray(
        w.reshape(NDC, 128, 8, 256).transpose(2, 1, 0, 3).astype(NP_DT))


def _pack_ws2(w):
    # [SH, D] -> [dq, p, hc, col]
    return np.ascontiguousarray(
        w.reshape(NHC, 128, NDQ, DW).transpose(2, 1, 0, 3).astype(NP_DT))


def _plan(xf, w_router):
    """fp32 routing + balanced token->core assignment.

    Returns (perm [NCORES, TC] token ids, caps [E]). Tokens are sorted by
    their (top1, top2) expert pair and dealt round-robin, so each core's
    per-expert counts are within a few of global/8.
    """
    logits = xf @ w_router
    part = np.argpartition(-logits, 2, axis=1)[:, :2]
    v = np.take_along_axis(logits, part, axis=1)
    sw = v[:, 0] < v[:, 1]
    part[sw] = part[sw][:, ::-1]
    pair = part[:, 0] * E + part[:, 1]
    order = np.argsort(pair, kind="stable")
    perm = order.reshape(TC, NCORES).T  # core c gets order[c::8]
    counts = np.zeros((NCORES, E), np.int64)
    for c in range(NCORES):
        np.add.at(counts[c], part[perm[c]].ravel(), 1)
    caps = [int(-(-(int(m) + CSLACK) // 16) * 16) for m in counts.max(axis=0)]
    caps = [min(c, CAPPAD) for c in caps]
    return perm, caps


def kernel(x, w_router, w1, w3, w2, ws1, ws3, ws2):
    x = np.asarray(x, dtype=np.float32)
    w_router = np.ascontiguousarray(np.asarray(w_router, dtype=np.float32))
    w1 = np.asarray(w1, dtype=np.float32)
    w3 = np.asarray(w3, dtype=np.float32)
    w2 = np.asarray(w2, dtype=np.float32) * (2.0 / 3.0)
    ws1 = np.asarray(ws1, dtype=np.float32)
    ws3 = np.asarray(ws3, dtype=np.float32)
    ws2 = np.asarray(ws2, dtype=np.float32) * (1.0 / 3.0)

    orig_shape = x.shape
    xf = np.ascontiguousarray(x.reshape(T, D))

    perm, caps = _plan(xf, w_router)

    idx = np.arange(128, dtype=np.float32)
    uts = (idx[:, None] < idx[None, :]).astype(np.float32)
    ones = np.ones((128, 128), dtype=np.float32)
    ecap2 = np.broadcast_to(
        np.arange(E, dtype=np.float32) * CAPPAD, (128, E))
    cpack = np.ascontiguousarray(
        np.concatenate([uts, ones, ecap2], axis=1, dtype=np.float32))
    tok = (np.arange(TC, dtype=np.int16).reshape(NTT, 128).T)[:, :, None]
    tok16 = np.ascontiguousarray(np.broadcast_to(tok, (128, NTT, 16)))
    invtab0 = np.full((E, NIC, 16, 16), DUMP, dtype=np.int16)
    wtab0 = np.zeros((E * CAPPAD, 16), dtype=np.float32)

    w1p, w3p = _pack_w13(w1), _pack_w13(w3)
    w2p = _pack_w2(w2)
    ws1p, ws3p = _pack_ws13(ws1), _pack_ws13(ws3)
    ws2p = _pack_ws2(ws2)
    wrp_h = np.ascontiguousarray(
        w_router.reshape(NDC, 128, E).transpose(1, 0, 2).astype(NP_DT))

    nc = _get_program(caps)

    in_maps = []
    for c in range(NCORES):
        xc = np.ascontiguousarray(xf[perm[c]])
        xtok = np.zeros((TC + 128, D), NP_DT)
        xtok[:TC] = xc.astype(NP_DT)
        xtrp = np.ascontiguousarray(
            xc.T.reshape(NDC, 128, TC).transpose(1, 0, 2).astype(NP_DT))
        in_maps.append({
            "x_tok": xtok, "x_trp": xtrp,
            "wrp": wrp_h,
            "w1p": w1p, "w3p": w3p, "w2p": w2p,
            "ws1p": ws1p, "ws3p": ws3p, "ws2p": ws2p,
            "cpack": cpack, "tok16": tok16,
            "invtab": invtab0, "wtab": wtab0,
        })

    res = run_bass_kernel_spmd(nc, in_maps, list(range(NCORES)))
    out = np.empty((T, D), np.float32)
    for c in range(NCORES):
        out[perm[c]] = res.results[c]["out"][:TC]
    return out.reshape(orig_shape).astype(np.float32)
